# revision 49
# baseline (speedup 1.0000x reference)
"""Trainium2 Bass kernel for nn_Attention (dense transformer attention block).

Reference semantics (B=2, S=2048, D=2048, NH=16, NKV=4, HD=128):
    qkv = x @ wqkv.T ; split q/k/v ; rmsnorm(q), rmsnorm(k) (weights == 1)
    rotary(q), rotary(k) with arbitrary freqs_cis ; GQA repeat kv 4x
    causal softmax attention ; out = y @ wo.T

Sharding: 2-way data parallel over batch x 4-way tensor parallel over head
groups (each core owns 4 query heads + their single shared KV head).  Each
core computes a full-shape partial of the output projection for its batch
element; the host sums the 4 partials per batch element (the "all-reduce").

Device layout notes:
  - All tensors are kept "transposed" (feature dim on partitions, tokens on
    the free dim) so every matmul chains without transposes; only V is
    transposed on-device (PE transpose) to the [token, dv] layout the
    attention-value matmul needs as its stationary operand.
  - Head dims are permuted (even dims then odd dims) on the host so rotary
    becomes two contiguous 64-partition slabs; the permutation cancels in
    the q.k contraction and V/wo are left in natural order.
  - Softmax uses no running max: |scores| <= sqrt(128)*max|f|^2 is far below
    exp overflow in fp32 (verified empirically), so exp() is applied
    directly and the denominator is accumulated with a ones-vector matmul.

Scheduling notes (this revision):
  - Phase 1 defers each chunk-tile's rmsnorm/rotary stats by one chunk-tile
    so the PE stream of projection matmuls never waits on the ACT
    square -> ones-matmul stats chain.
  - x streams in token-tile-major order so the first projection tile's
    inputs land first (startup ~7us instead of ~16us).
  - Attention runs two head-units in a software pipeline at key-tile grain:
    the PE queue is [score_A(i+1), dps_A(i), yps_A(i), score_B(i+1), ...]
    so PE stays busy while ACT computes the exps one tile behind.
  - The causal mask is a PSUM preload (0 / -1e30 bias written by DVE before
    the score matmul accumulates onto it) instead of a DVE multiply inside
    the score -> exp -> accumulate chain.
  - Attention outputs are copied out of PSUM unnormalized (frees the bank
    for the next pair immediately) and normalized in place in SBUF once the
    reciprocal broadcast lands.
  - Out-projection blocks are emitted as PE filler between the attention
    iterations of the next query tile; output partials are stored bf16
    (the host sums in f32).
"""

import math
from collections import deque
from contextlib import ExitStack

import numpy as np

B, S, D = 2, 2048, 2048
NH, NKV, HD = 16, 4, 128
EPS = 1e-6
N_CORES = 8
TPC = 4            # tensor-parallel cores per batch element
HEADS_PER_CORE = NH // TPC          # 4
Q_SIZE, KV_SIZE = NH * HD, NKV * HD
E_LOC = HEADS_PER_CORE * HD         # 512 local y/e dims per core
TT = 512                            # token tile (free dim) for matmuls
N_TT = S // TT                      # 4
N_KT = D // 128                     # 16 contraction tiles for projections
N_SKT = S // 128                    # 16 key tiles per sequence

_F32 = "float32"
# causal-mask bias: (score + NEG) * SCALE ~ -177 keeps the ACT exp table in
# a sane input range (a -1e30 bias produced garbage on hardware even though
# CoreSim's exact exp gave 0) while exp underflows to 0 in fp32.
NEG = -2000.0


def _steer_act_tables():
    """Make Exp and Ln both resolve to the combined natural_log_exp table.

    bacc's insert_act_table_loads picks the first act-function set that
    contains each function, which puts Exp and Ln in different tables and
    costs a ~1.3us ACT table re-load on every rmsnorm <-> softmax switch
    (measured 40 loads / 51us per core).  Stripping Exp/Ln from the other
    sets (list positions preserved, so set ids stay valid for walrus)
    leaves one shared table and a single load.
    """
    from concourse import bacc
    import concourse.mybir as mybir
    import concourse.hw_specs as hw_specs

    if getattr(bacc.get_activation_tables, "_act_steered", False):
        return
    orig = hw_specs.get_activation_tables

    def steered(arch):
        tabs = orig(arch)
        for name, fns in tabs.items():
            if name != "natural_log_exp_and_others":
                fns.discard(mybir.ActivationFunctionType.Exp)
                fns.discard(mybir.ActivationFunctionType.Ln)
        return tabs

    steered._act_steered = True
    bacc.get_activation_tables = steered


def _build_bass():
    import concourse.bass as bass  # noqa: F401
    import concourse.mybir as mybir
    import concourse.tile as tile
    from concourse import bacc
    from concourse.masks import make_identity

    _steer_act_tables()

    f32 = mybir.dt.float32
    bf16 = mybir.dt.bfloat16

    nc = bacc.Bacc("TRN2", target_bir_lowering=False, debug=False,
                   num_devices=N_CORES)

    # ---- DRAM I/O (per-core shards supplied via in_maps) ----
    xT_d = nc.dram_tensor("xT", (D, S), bf16, kind="ExternalInput").ap()
    # per-chunk-contiguous weight layout: [chunk, p, ko, e] so each chunk's
    # stationary tiles stream in with 4 KiB/partition contiguous lines
    wqkvT_d = nc.dram_tensor(
        "wqkvT", (HEADS_PER_CORE + 2, 128, N_KT, HD), bf16,
        kind="ExternalInput").ap()
    woT_d = nc.dram_tensor("woT", (E_LOC, D), bf16, kind="ExternalInput").ap()
    # fr/fi are duplicated across both 64-partition halves so rotary ops can
    # pair them with either the even (base 0) or odd (base 64) slab of q/k
    fr_d = nc.dram_tensor("fr", (HD, S), f32, kind="ExternalInput").ap()
    fi_d = nc.dram_tensor("fi", (HD, S), f32, kind="ExternalInput").ap()
    # causal bias for the leading 128-col diagonal block of a score tile is
    # produced BY a matmul (mtri.T @ mshift = NEG where key > query): only
    # TensorE writes touch PSUM's has_written bits, so a DVE preload would
    # be overwritten by the accumulating score matmul (documented gotcha).
    mtri_d = nc.dram_tensor("mtri", (128, 128), bf16,
                            kind="ExternalInput").ap()
    mshift_d = nc.dram_tensor("mshift", (128, 128), bf16,
                              kind="ExternalInput").ap()
    outT_d = nc.dram_tensor("outT", (D, S), bf16, kind="ExternalOutput").ap()

    KCH = HEADS_PER_CORE          # k chunk index
    VCH = HEADS_PER_CORE + 1      # v chunk index
    SCALE = 1.0 / math.sqrt(HD)

    with tile.TileContext(nc) as tc, ExitStack() as ctx:
        # ---------- pools ----------
        const = ctx.enter_context(tc.tile_pool(name="const", bufs=1))
        sb = ctx.enter_context(tc.tile_pool(name="sb", bufs=2))
        epool = ctx.enter_context(tc.tile_pool(name="epool", bufs=6))
        psum = ctx.enter_context(tc.tile_pool(name="psum", bufs=2,
                                              space="PSUM"))
        pproj = ctx.enter_context(tc.tile_pool(name="pproj", bufs=2,
                                               space="PSUM"))
        pacc = ctx.enter_context(tc.tile_pool(name="pacc", bufs=2,
                                              space="PSUM"))
        prow = ctx.enter_context(tc.tile_pool(name="prow", bufs=2,
                                              space="PSUM"))

        # ---------- resident tensors ----------
        # phase-1-only tensors live in their own pool, freed before attention
        # needs peak SBUF
        p1_ctx = ExitStack()
        p1 = p1_ctx.enter_context(tc.tile_pool(name="p1", bufs=1))
        p1w = p1_ctx.enter_context(tc.tile_pool(name="p1w", bufs=6))
        xT = p1.tile([128, N_KT, S], bf16)               # 64 KiB/part
        xT_r = xT_d.rearrange("(ko p) t -> p ko t", p=128)
        fr = p1.tile([HD, S], f32)
        fi = p1.tile([HD, S], f32)
        woT = const.tile([128, HEADS_PER_CORE, D], bf16)
        mtri = const.tile([128, 128], bf16)
        mshift = const.tile([128, 128], bf16)

        def load_wch(chunk):
            wch = p1w.tile([128, N_KT, HD], bf16, tag="wch", name="wch")
            nc.sync.dma_start(wch[:], wqkvT_d[chunk])
            return wch

        # DMA priority: everything on ONE queue, strictly in consumption
        # order, so early transfers are never starved by bulk constants.
        # The first chunk-tile needs wch(k) + x[tt0]; fr/fi gate the first
        # stats; the remaining weights/x stream ahead of their matmuls;
        # mask/wo are not needed until attention/out-proj.
        wtiles = {KCH: load_wch(KCH)}
        ts0 = slice(0, TT)
        nc.sync.dma_start(xT[:, :, ts0], xT_r[:, :, ts0])
        nc.sync.dma_start(fr[:], fr_d)
        nc.sync.dma_start(fi[:], fi_d)
        wtiles[VCH] = load_wch(VCH)
        wtiles[0] = load_wch(0)
        ts = slice(TT, 2 * TT)
        nc.sync.dma_start(xT[:, :, ts], xT_r[:, :, ts])
        for chunk in (1, 2, 3):
            wtiles[chunk] = load_wch(chunk)
        for tt in range(2, N_TT):
            ts = slice(tt * TT, (tt + 1) * TT)
            nc.sync.dma_start(xT[:, :, ts], xT_r[:, :, ts])
        nc.sync.dma_start(mtri[:], mtri_d)
        nc.sync.dma_start(mshift[:], mshift_d)
        nc.sync.dma_start(
            woT[:], woT_d.rearrange("(eo p) d -> p eo d", p=128))

        ident = const.tile([128, 128], bf16)
        make_identity(nc, ident[:])
        # all-ones stationary: matmul(out, onesm, e) puts the column sums of
        # e in EVERY output partition -- a partition-reduce and broadcast in
        # one full-rate matmul
        onesm = const.tile([128, 128], bf16)
        nc.vector.memset(onesm[:], 1.0)
        epsb = const.tile([128, 1], f32)
        nc.vector.memset(epsb[:], EPS)

        # rotated q (4 heads), rotated k, and v in [token, dv] layout
        qrot = [const.tile([128, S], bf16, tag=f"qrot{h}", name=f"qrot{h}")
                for h in range(HEADS_PER_CORE)]
        krot = const.tile([128, S], bf16)
        vT = const.tile([128, S], bf16)
        vtok = const.tile([128, N_SKT, HD], bf16)
        # attention outputs (yT), stationary input of out-proj; written
        # unnormalized then scaled in place
        yT = [const.tile([128, S], bf16, tag=f"yT{h}", name=f"yT{h}")
              for h in range(HEADS_PER_CORE)]

        # ---------- phase 1: QKV projection (+norm+rotary), software
        # pipelined so the stats chain of chunk-tile j runs while the PE
        # streams the matmuls of chunk-tile j+1 ----------
        def emit_mm(chunk, tt):
            ts = slice(tt * TT, (tt + 1) * TT)
            wch = wtiles[chunk]
            ps = pproj.tile([128, TT], f32, tag="proj", name="ps")
            for kt in range(N_KT):
                nc.tensor.matmul(
                    ps[:], wch[:, kt, :],
                    xT[:, kt, ts], start=(kt == 0), stop=(kt == N_KT - 1))
            return ps

        def emit_stats(chunk, tt, ps):
            ts = slice(tt * TT, (tt + 1) * TT)
            if chunk == VCH:
                nc.vector.tensor_copy(vT[:, ts], ps[:])
                return
            # rms stats: mean over head dim (partitions) via all-ones
            # matmul, which lands the sums pre-broadcast in all 128
            # partitions; ACT ops are column-parallel so [128, TT] ln/exp
            # cost the same as a [1, TT] row would
            sq = sb.tile([128, TT], bf16, tag="sq", name="sq")
            nc.scalar.activation(sq[:], ps[:],
                                 mybir.ActivationFunctionType.Square)
            ms = prow.tile([128, TT], f32, tag="rowp", name="ms")
            nc.tensor.matmul(ms[:], onesm[:], sq[:], start=True, stop=True)
            lnms = sb.tile([128, TT], f32, tag="lnms", name="lnms")
            nc.scalar.activation(lnms[:], ms[:],
                                 mybir.ActivationFunctionType.Ln,
                                 bias=epsb[:], scale=1.0 / HD)
            rsb = sb.tile([128, TT], f32, tag="rsb", name="rsb")
            nc.scalar.activation(rsb[:], lnms[:],
                                 mybir.ActivationFunctionType.Exp,
                                 bias=0.0, scale=-0.5)
            # rotary, even dims on partitions 0:64, odd on 64:128:
            #   a      = q * fr            (both halves at once)
            #   bswap  = swap_halves(q) * [+fi; -fi]  (2 cross-half muls;
            #            the sign baked into fi makes the combine an add)
            #   rot    = a + bswap
            rot = sb.tile([128, TT], f32, tag="rot", name="rot")
            a = sb.tile([128, TT], f32, tag="rota", name="a")
            nc.vector.tensor_mul(a[:], ps[:], fr[:, ts])
            bsw = sb.tile([128, TT], f32, tag="rotb", name="bsw")
            nc.vector.tensor_mul(bsw[0:64, :], ps[64:128, :],
                                 fi[64:128, ts])
            nc.vector.tensor_mul(bsw[64:128, :], ps[0:64, :],
                                 fi[0:64, ts])
            nc.vector.tensor_add(rot[:], a[:], bsw[:])
            dst = krot if chunk == KCH else qrot[chunk]
            nc.vector.tensor_mul(dst[:, ts], rot[:], rsb[:])

        def emit_vtrans():
            for i in range(N_SKT):             # v -> [token, dv] layout
                tp = psum.tile([128, 128], bf16, tag="mm", name="tp")
                nc.tensor.transpose(tp[:], vT[:, i * 128:(i + 1) * 128],
                                    ident[:])
                nc.vector.tensor_copy(vtok[:, i, :], tp[:])

        # deferred-work queue, pumped between attention iterations: holds
        # the tail rmsnorm stats (so the first attention exps are not
        # queued behind them on ACT) and later the out-proj blocks
        filler = deque()

        def pump(n=2):
            for _ in range(min(n, len(filler))):
                filler.popleft()()

        # tt-major across chunks: once the first 2 MiB x token-tile lands,
        # the PE has all six chunks' matmuls for it (~20us of work) while
        # the remaining x tiles stream in
        chunk_order = [KCH, VCH, 0, 1, 2, 3]
        jobs = [(c, tt) for tt in range(N_TT) for c in chunk_order]
        pending = None                 # (chunk, tt, ps) awaiting stats
        for ji, (chunk, tt) in enumerate(jobs):
            ps = emit_mm(chunk, tt)
            if pending is not None:
                emit_stats(*pending)
                if pending[0] == VCH and pending[1] == N_TT - 1:
                    emit_vtrans()
            pending = (chunk, tt, ps)
        emit_stats(*pending)
        p1_ctx.close()   # xT/wqkvT/fr/fi no longer needed

        # ---------- phase 2/3: attention pairs + out-proj filler ----------

        def outproj_block(qt, m):
            def go():
                qs = slice(qt * TT, (qt + 1) * TT)
                ops = pproj.tile([128, TT], f32, tag="proj", name="ops")
                for e in range(HEADS_PER_CORE):
                    nc.tensor.matmul(ops[:],
                                     woT[:, e, m * 128:(m + 1) * 128],
                                     yT[e][:, qs], start=(e == 0),
                                     stop=(e == HEADS_PER_CORE - 1))
                osb = sb.tile([128, TT], bf16, tag="osb", name="osb")
                nc.vector.tensor_copy(osb[:], ops[:])
                nc.sync.dma_start(outT_d[m * 128:(m + 1) * 128, qs], osb[:])
            return go

        def emit_pair(hA, hB, qt):
            ntk = 4 * (qt + 1)
            units = []
            for ui, h in enumerate((hA, hB)):
                units.append({
                    "h": h,
                    "dps": prow.tile([128, TT], f32, tag="rowp",
                                     name=f"dps{h}"),
                    "yps": pacc.tile([128, TT], f32, tag="yacc",
                                     name=f"yps{h}"),
                    "sps": [None] * ntk,
                    "e": [None] * ntk,
                })

            def emit_score(u, tk):
                # diagonal tiles (r >= 0) only have valid scores in their
                # last TT - 128*r columns; skip the fully-masked prefix.
                # In suffix-local coords the causal triangle is always the
                # first 128 columns; it gets a -1e30 PSUM preload so exp
                # masks for free.
                r = tk - 4 * qt
                off = 128 * r if r > 0 else 0
                w = TT - off
                q0 = qt * TT + off
                sps = psum.tile([128, TT], f32, tag="mm", name="sps")
                u["sps"][tk] = (sps, off, w)
                kblk = krot[:, tk * 128:(tk + 1) * 128]
                qh = qrot[u["h"]]
                if r >= 0:
                    # mask bias comes from the PE itself (has_written gotcha)
                    nc.tensor.matmul(sps[:, 0:128], mtri[:], mshift[:],
                                     start=True, stop=False)
                    nc.tensor.matmul(sps[:, 0:128], kblk,
                                     qh[:, q0:q0 + 128],
                                     start=False, stop=(w == 128))
                    if w > 128:
                        nc.tensor.matmul(sps[:, 128:w], kblk,
                                         qh[:, q0 + 128:q0 + w],
                                         start=False, stop=True)
                else:
                    nc.tensor.matmul(sps[:, :w], kblk, qh[:, q0:q0 + w],
                                     start=True, stop=True)

            def emit_exp(u, tk):
                sps, off, w = u["sps"][tk]
                e = epool.tile([128, TT], bf16, tag="e", name="e")
                nc.scalar.activation(e[:, :w], sps[:, :w],
                                     mybir.ActivationFunctionType.Exp,
                                     bias=0.0, scale=SCALE)
                u["sps"][tk] = None
                u["e"][tk] = (e, off, w)

            def emit_acc(u, tk):
                e, off, w = u["e"][tk]
                nc.tensor.matmul(u["dps"][:, off:], onesm[:], e[:, :w],
                                 start=(tk == 0), stop=(tk == ntk - 1))
                nc.tensor.matmul(u["yps"][:, off:], vtok[:, tk, :],
                                 e[:, :w],
                                 start=(tk == 0), stop=(tk == ntk - 1))
                u["e"][tk] = None

            for u in units:                      # prologue
                emit_score(u, 0)
                emit_exp(u, 0)
            for i in range(ntk):
                for u in units:
                    if i + 1 < ntk:
                        emit_score(u, i + 1)
                        emit_exp(u, i + 1)
                    emit_acc(u, i)
                pump()
            qs = slice(qt * TT, (qt + 1) * TT)
            for u in units:                      # epilogue
                # dps holds the denominator replicated in all partitions;
                # reciprocal + one fused normalize-and-store multiply
                drb = sb.tile([128, TT], f32, tag="drb", name="drb")
                nc.vector.reciprocal_approx_fast(drb[:], u["dps"][:])
                nc.vector.tensor_mul(yT[u["h"]][:, qs], u["yps"][:],
                                     drb[:])

        for qt in (3, 2, 1, 0):
            emit_pair(0, 1, qt)
            emit_pair(2, 3, qt)
            filler.extend(outproj_block(qt, m) for m in range(D // 128))
        while filler:
            filler.popleft()()

    nc.compile()
    return nc


def _host_shards(x, freqs_cis, wqkv, wo):
    import ml_dtypes
    bf16 = ml_dtypes.bfloat16

    # head-dim permutation: even dims then odd dims (for q and k only)
    perm = np.concatenate([np.arange(0, HD, 2), np.arange(1, HD, 2)])

    wq = wqkv[:Q_SIZE].reshape(NH, HD, D)[:, perm, :]
    wk = wqkv[Q_SIZE:Q_SIZE + KV_SIZE].reshape(NKV, HD, D)[:, perm, :]
    wv = wqkv[Q_SIZE + KV_SIZE:].reshape(NKV, HD, D)

    fr1 = np.ascontiguousarray(freqs_cis[:, :, 0].T, dtype=np.float32)
    fi1 = np.ascontiguousarray(freqs_cis[:, :, 1].T, dtype=np.float32)
    fr = np.vstack([fr1, fr1])
    # sign baked in so the rotary combine is a single add:
    #   rot[lo] = q_lo*fr + q_hi*(-fi) ; rot[hi] = q_hi*fr + q_lo*(+fi)
    fi = np.vstack([fi1, -fi1])

    # mask matmul operands: (mtri.T @ mshift)[p, c] = NEG iff p > c
    # (suffix-local causal bias for the diagonal 128x128 score block)
    mtri = np.triu(np.ones((128, 128), np.float32)).astype(bf16)
    mshift = (NEG * np.eye(128, k=-1, dtype=np.float32)).astype(bf16)

    in_maps = []
    for c in range(N_CORES):
        b, j = divmod(c, TPC)
        wshard = np.concatenate(
            [wq[TPC * j + h] for h in range(HEADS_PER_CORE)] +
            [wk[j], wv[j]], axis=0)                     # (768, D)
        # [chunk, p, ko, e] with d = ko*128 + p
        wpack = np.ascontiguousarray(
            wshard.reshape(HEADS_PER_CORE + 2, HD, N_KT, 128)
            .transpose(0, 3, 2, 1)).astype(bf16)
        in_maps.append({
            "xT": np.ascontiguousarray(x[b].T).astype(bf16),
            "wqkvT": wpack,
            "woT": np.ascontiguousarray(
                wo[:, j * E_LOC:(j + 1) * E_LOC].T).astype(bf16),
            "fr": fr,
            "fi": fi,
            "mtri": mtri,
            "mshift": mshift,
        })
    return in_maps


_NC_CACHE = {}


def _get_nc():
    if "nc" not in _NC_CACHE:
        _NC_CACHE["nc"] = _build_bass()
    return _NC_CACHE["nc"]


def kernel(x, freqs_cis, wqkv, wo, q_norm_w, k_norm_w, _want_results=False):
    # q_norm_w / k_norm_w are all-ones per the problem spec; rmsnorm weight
    # multiply is the identity and is folded away.
    from concourse.bass_utils import run_bass_kernel_spmd

    nc = _get_nc()
    in_maps = _host_shards(np.asarray(x, np.float32),
                           np.asarray(freqs_cis, np.float32),
                           np.asarray(wqkv, np.float32),
                           np.asarray(wo, np.float32))
    res = run_bass_kernel_spmd(nc, in_maps, core_ids=list(range(N_CORES)))
    parts = [r["outT"] for r in res.results]
    out = np.empty((B, S, D), np.float32)
    for b in range(B):
        acc = parts[TPC * b].astype(np.float32)
        for j in range(1, TPC):
            acc += parts[TPC * b + j].astype(np.float32)
        out[b] = acc.T
    if _want_results:
        return out, res
    return out


# revision 52
# speedup vs baseline: 1.2360x; 1.2360x over previous
"""Trainium2 Bass kernel for nn_Attention (dense transformer attention block).

Reference semantics (B=2, S=2048, D=2048, NH=16, NKV=4, HD=128):
    qkv = x @ wqkv.T ; split q/k/v ; rmsnorm(q), rmsnorm(k) (weights == 1)
    rotary(q), rotary(k) with arbitrary freqs_cis ; GQA repeat kv 4x
    causal softmax attention ; out = y @ wo.T

Sharding: 2-way data parallel over batch x 4-way tensor parallel over head
groups (each core owns 4 query heads + their single shared KV head).  Each
core computes a full-shape partial of the output projection for its batch
element; the host sums the 4 partials per batch element (the "all-reduce").

Device layout notes:
  - All tensors are kept "transposed" (feature dim on partitions, tokens on
    the free dim) so every matmul chains without transposes; only V is
    transposed on-device (PE transpose) to the [token, dv] layout the
    attention-value matmul needs as its stationary operand.
  - Head dims are permuted (even dims then odd dims) on the host so rotary
    becomes two contiguous 64-partition slabs; the permutation cancels in
    the q.k contraction and V/wo are left in natural order.
  - Softmax uses no running max: |scores| <= sqrt(128)*max|f|^2 is far below
    exp overflow in fp32 (verified empirically), so exp() is applied
    directly and the denominator is accumulated with a ones-vector matmul.

Scheduling notes (this revision):
  - Phase 1 defers each chunk-tile's rmsnorm/rotary stats by one chunk-tile
    so the PE stream of projection matmuls never waits on the ACT
    square -> ones-matmul stats chain.
  - x streams in token-tile-major order so the first projection tile's
    inputs land first (startup ~7us instead of ~16us).
  - Attention runs two head-units in a software pipeline at key-tile grain:
    the PE queue is [score_A(i+1), dps_A(i), yps_A(i), score_B(i+1), ...]
    so PE stays busy while ACT computes the exps one tile behind.
  - The causal mask is a PSUM preload (0 / -1e30 bias written by DVE before
    the score matmul accumulates onto it) instead of a DVE multiply inside
    the score -> exp -> accumulate chain.
  - Attention outputs are copied out of PSUM unnormalized (frees the bank
    for the next pair immediately) and normalized in place in SBUF once the
    reciprocal broadcast lands.
  - Out-projection blocks are emitted as PE filler between the attention
    iterations of the next query tile; output partials are stored bf16
    (the host sums in f32).
"""

import math
from collections import deque
from contextlib import ExitStack

import numpy as np

B, S, D = 2, 2048, 2048
NH, NKV, HD = 16, 4, 128
EPS = 1e-6
N_CORES = 8
TPC = 4            # tensor-parallel cores per batch element
HEADS_PER_CORE = NH // TPC          # 4
Q_SIZE, KV_SIZE = NH * HD, NKV * HD
E_LOC = HEADS_PER_CORE * HD         # 512 local y/e dims per core
TT = 512                            # token tile (free dim) for matmuls
N_TT = S // TT                      # 4
N_KT = D // 128                     # 16 contraction tiles for projections
N_SKT = S // 128                    # 16 key tiles per sequence

_F32 = "float32"
# causal-mask bias: (score + NEG) * SCALE ~ -177 keeps the ACT exp table in
# a sane input range (a -1e30 bias produced garbage on hardware even though
# CoreSim's exact exp gave 0) while exp underflows to 0 in fp32.
NEG = -2000.0


def _steer_act_tables():
    """Make Exp and Ln both resolve to the combined natural_log_exp table.

    bacc's insert_act_table_loads picks the first act-function set that
    contains each function, which puts Exp and Ln in different tables and
    costs a ~1.3us ACT table re-load on every rmsnorm <-> softmax switch
    (measured 40 loads / 51us per core).  Stripping Exp/Ln from the other
    sets (list positions preserved, so set ids stay valid for walrus)
    leaves one shared table and a single load.
    """
    from concourse import bacc
    import concourse.mybir as mybir
    import concourse.hw_specs as hw_specs

    if getattr(bacc.get_activation_tables, "_act_steered", False):
        return
    orig = hw_specs.get_activation_tables

    def steered(arch):
        tabs = orig(arch)
        for name, fns in tabs.items():
            if name != "natural_log_exp_and_others":
                fns.discard(mybir.ActivationFunctionType.Exp)
                fns.discard(mybir.ActivationFunctionType.Ln)
        return tabs

    steered._act_steered = True
    bacc.get_activation_tables = steered


def _build_bass():
    import concourse.bass as bass  # noqa: F401
    import concourse.mybir as mybir
    import concourse.tile as tile
    from concourse import bacc
    from concourse.masks import make_identity

    _steer_act_tables()

    f32 = mybir.dt.float32
    bf16 = mybir.dt.bfloat16

    nc = bacc.Bacc("TRN2", target_bir_lowering=False, debug=False,
                   num_devices=N_CORES)

    # ---- DRAM I/O (per-core shards supplied via in_maps) ----
    xT_d = nc.dram_tensor("xT", (D, S), bf16, kind="ExternalInput").ap()
    # per-chunk-contiguous weight layout: [chunk, p, ko, e] so each chunk's
    # stationary tiles stream in with 4 KiB/partition contiguous lines
    wqkvT_d = nc.dram_tensor(
        "wqkvT", (HEADS_PER_CORE + 2, 128, N_KT, HD), bf16,
        kind="ExternalInput").ap()
    woT_d = nc.dram_tensor("woT", (E_LOC, D), bf16, kind="ExternalInput").ap()
    # fr/fi are duplicated across both 64-partition halves so rotary ops can
    # pair them with either the even (base 0) or odd (base 64) slab of q/k
    fr_d = nc.dram_tensor("fr", (HD, S), f32, kind="ExternalInput").ap()
    fi_d = nc.dram_tensor("fi", (HD, S), f32, kind="ExternalInput").ap()
    # causal bias for the leading 128-col diagonal block of a score tile is
    # produced BY a matmul (mtri.T @ mshift = NEG where key > query): only
    # TensorE writes touch PSUM's has_written bits, so a DVE preload would
    # be overwritten by the accumulating score matmul (documented gotcha).
    mtri_d = nc.dram_tensor("mtri", (128, 128), bf16,
                            kind="ExternalInput").ap()
    mshift_d = nc.dram_tensor("mshift", (128, 128), bf16,
                              kind="ExternalInput").ap()
    outT_d = nc.dram_tensor("outT", (D, S), bf16, kind="ExternalOutput").ap()

    KCH = HEADS_PER_CORE          # k chunk index
    VCH = HEADS_PER_CORE + 1      # v chunk index
    SCALE = 1.0 / math.sqrt(HD)

    with tile.TileContext(nc) as tc, ExitStack() as ctx:
        # ---------- pools ----------
        const = ctx.enter_context(tc.tile_pool(name="const", bufs=1))
        sb = ctx.enter_context(tc.tile_pool(name="sb", bufs=2))
        # out-proj staging: deep ring so a PSUM->SBUF copy never waits on
        # the store-DMA completion (+900ns sem) of the block two back
        osbp = ctx.enter_context(tc.tile_pool(name="osbp", bufs=6))
        epool = ctx.enter_context(tc.tile_pool(name="epool", bufs=6))
        psum = ctx.enter_context(tc.tile_pool(name="psum", bufs=2,
                                              space="PSUM"))
        pproj = ctx.enter_context(tc.tile_pool(name="pproj", bufs=2,
                                               space="PSUM"))
        pacc = ctx.enter_context(tc.tile_pool(name="pacc", bufs=2,
                                              space="PSUM"))
        prow = ctx.enter_context(tc.tile_pool(name="prow", bufs=2,
                                              space="PSUM"))

        # ---------- resident tensors ----------
        # phase-1-only tensors live in their own pool, freed before attention
        # needs peak SBUF
        p1_ctx = ExitStack()
        p1 = p1_ctx.enter_context(tc.tile_pool(name="p1", bufs=1))
        p1w = p1_ctx.enter_context(tc.tile_pool(name="p1w", bufs=6))
        xT = p1.tile([128, N_KT, S], bf16)               # 64 KiB/part
        xT_r = xT_d.rearrange("(ko p) t -> p ko t", p=128)
        fr = p1.tile([HD, S], f32)
        fi = p1.tile([HD, S], f32)
        woT = const.tile([128, HEADS_PER_CORE, D], bf16)
        mtri = const.tile([128, 128], bf16)
        mshift = const.tile([128, 128], bf16)

        def load_wch(chunk):
            wch = p1w.tile([128, N_KT, HD], bf16, tag="wch", name="wch")
            nc.sync.dma_start(wch[:], wqkvT_d[chunk])
            return wch

        # DMA priority: everything on ONE queue, strictly in consumption
        # order, so early transfers are never starved by bulk constants.
        # The first chunk-tile needs wch(k) + x[tt0]; fr/fi gate the first
        # stats; the remaining weights/x stream ahead of their matmuls;
        # mask/wo are not needed until attention/out-proj.
        wtiles = {KCH: load_wch(KCH)}
        ts0 = slice(0, TT)
        nc.sync.dma_start(xT[:, :, ts0], xT_r[:, :, ts0])
        nc.sync.dma_start(fr[:], fr_d)
        nc.sync.dma_start(fi[:], fi_d)
        wtiles[VCH] = load_wch(VCH)
        wtiles[0] = load_wch(0)
        ts = slice(TT, 2 * TT)
        nc.sync.dma_start(xT[:, :, ts], xT_r[:, :, ts])
        for chunk in (1, 2, 3):
            wtiles[chunk] = load_wch(chunk)
        for tt in range(2, N_TT):
            ts = slice(tt * TT, (tt + 1) * TT)
            nc.sync.dma_start(xT[:, :, ts], xT_r[:, :, ts])
        nc.sync.dma_start(mtri[:], mtri_d)
        nc.sync.dma_start(mshift[:], mshift_d)
        nc.sync.dma_start(
            woT[:], woT_d.rearrange("(eo p) d -> p eo d", p=128))

        ident = const.tile([128, 128], bf16)
        make_identity(nc, ident[:])
        # all-ones stationary: matmul(out, onesm, e) puts the column sums of
        # e in EVERY output partition -- a partition-reduce and broadcast in
        # one full-rate matmul
        onesm = const.tile([128, 128], bf16)
        nc.vector.memset(onesm[:], 1.0)
        epsb = const.tile([128, 1], f32)
        nc.vector.memset(epsb[:], EPS)
        # prime the Exp/Ln activation table during the initial DMA wait so
        # the first rmsnorm stats chain doesn't pay the ~1.3us table load
        scr = const.tile([1, 1], f32)
        nc.vector.memset(scr[:], 1.0)
        nc.scalar.activation(scr[:], scr[:],
                             mybir.ActivationFunctionType.Exp,
                             bias=0.0, scale=0.0)

        # rotated q (4 heads), rotated k, and v in [token, dv] layout
        qrot = [const.tile([128, S], bf16, tag=f"qrot{h}", name=f"qrot{h}")
                for h in range(HEADS_PER_CORE)]
        krot = const.tile([128, S], bf16)
        vT = const.tile([128, S], bf16)
        vtok = const.tile([128, N_SKT, HD], bf16)
        # attention outputs (yT), stationary input of out-proj; written
        # unnormalized then scaled in place
        yT = [const.tile([128, S], bf16, tag=f"yT{h}", name=f"yT{h}")
              for h in range(HEADS_PER_CORE)]

        # ---------- phase 1: QKV projection (+norm+rotary), software
        # pipelined so the stats chain of chunk-tile j runs while the PE
        # streams the matmuls of chunk-tile j+1 ----------
        def emit_mm(chunk, tt):
            ts = slice(tt * TT, (tt + 1) * TT)
            wch = wtiles[chunk]
            ps = pproj.tile([128, TT], f32, tag="proj", name="ps")
            for kt in range(N_KT):
                nc.tensor.matmul(
                    ps[:], wch[:, kt, :],
                    xT[:, kt, ts], start=(kt == 0), stop=(kt == N_KT - 1))
            return ps

        def emit_stats(chunk, tt, ps):
            ts = slice(tt * TT, (tt + 1) * TT)
            if chunk == VCH:
                nc.vector.tensor_copy(vT[:, ts], ps[:])
                return
            # rms stats: mean over head dim (partitions) via all-ones
            # matmul, which lands the sums pre-broadcast in all 128
            # partitions; ACT ops are column-parallel so [128, TT] ln/exp
            # cost the same as a [1, TT] row would
            sq = sb.tile([128, TT], bf16, tag="sq", name="sq")
            nc.scalar.activation(sq[:], ps[:],
                                 mybir.ActivationFunctionType.Square)
            ms = prow.tile([128, TT], f32, tag="rowp", name="ms")
            nc.tensor.matmul(ms[:], onesm[:], sq[:], start=True, stop=True)
            lnms = sb.tile([128, TT], f32, tag="lnms", name="lnms")
            nc.scalar.activation(lnms[:], ms[:],
                                 mybir.ActivationFunctionType.Ln,
                                 bias=epsb[:], scale=1.0 / HD)
            rsb = sb.tile([128, TT], f32, tag="rsb", name="rsb")
            nc.scalar.activation(rsb[:], lnms[:],
                                 mybir.ActivationFunctionType.Exp,
                                 bias=0.0, scale=-0.5)
            # rotary, even dims on partitions 0:64, odd on 64:128:
            #   a      = q * fr            (both halves at once)
            #   bswap  = swap_halves(q) * [+fi; -fi]  (2 cross-half muls;
            #            the sign baked into fi makes the combine an add)
            #   rot    = a + bswap
            rot = sb.tile([128, TT], f32, tag="rot", name="rot")
            a = sb.tile([128, TT], f32, tag="rota", name="a")
            nc.vector.tensor_mul(a[:], ps[:], fr[:, ts])
            bsw = sb.tile([128, TT], f32, tag="rotb", name="bsw")
            nc.vector.tensor_mul(bsw[0:64, :], ps[64:128, :],
                                 fi[64:128, ts])
            nc.vector.tensor_mul(bsw[64:128, :], ps[0:64, :],
                                 fi[0:64, ts])
            nc.vector.tensor_add(rot[:], a[:], bsw[:])
            dst = krot if chunk == KCH else qrot[chunk]
            nc.vector.tensor_mul(dst[:, ts], rot[:], rsb[:])

        def emit_vtrans():
            for i in range(N_SKT):             # v -> [token, dv] layout
                tp = psum.tile([128, 128], bf16, tag="mm", name="tp")
                nc.tensor.transpose(tp[:], vT[:, i * 128:(i + 1) * 128],
                                    ident[:])
                nc.vector.tensor_copy(vtok[:, i, :], tp[:])

        # deferred-work queue, pumped between attention iterations: holds
        # the tail rmsnorm stats (so the first attention exps are not
        # queued behind them on ACT) and later the out-proj blocks
        filler = deque()

        def pump(n=2):
            for _ in range(min(n, len(filler))):
                filler.popleft()()

        # tt-major across chunks: once the first 2 MiB x token-tile lands,
        # the PE has all six chunks' matmuls for it (~20us of work) while
        # the remaining x tiles stream in
        chunk_order = [KCH, VCH, 0, 1, 2, 3]
        jobs = [(c, tt) for tt in range(N_TT) for c in chunk_order]
        pending = None                 # (chunk, tt, ps) awaiting stats
        for ji, (chunk, tt) in enumerate(jobs):
            ps = emit_mm(chunk, tt)
            if pending is not None:
                emit_stats(*pending)
                if pending[0] == VCH and pending[1] == N_TT - 1:
                    emit_vtrans()
            pending = (chunk, tt, ps)
        emit_stats(*pending)
        p1_ctx.close()   # xT/wqkvT/fr/fi no longer needed

        # ---------- phase 2/3: attention pairs + out-proj filler ----------

        def outproj_block(qt, m):
            def go():
                qs = slice(qt * TT, (qt + 1) * TT)
                ops = pproj.tile([128, TT], f32, tag="proj", name="ops")
                for e in range(HEADS_PER_CORE):
                    nc.tensor.matmul(ops[:],
                                     woT[:, e, m * 128:(m + 1) * 128],
                                     yT[e][:, qs], start=(e == 0),
                                     stop=(e == HEADS_PER_CORE - 1))
                osb = osbp.tile([128, TT], bf16, tag="osb", name="osb")
                nc.vector.tensor_copy(osb[:], ops[:])
                nc.sync.dma_start(outT_d[m * 128:(m + 1) * 128, qs], osb[:])
            return go

        def emit_pair(hA, hB, qt):
            ntk = 4 * (qt + 1)
            units = []
            for ui, h in enumerate((hA, hB)):
                units.append({
                    "h": h,
                    "dps": prow.tile([128, TT], f32, tag="rowp",
                                     name=f"dps{h}"),
                    "yps": pacc.tile([128, TT], f32, tag="yacc",
                                     name=f"yps{h}"),
                    "sps": [None] * ntk,
                    "e": [None] * ntk,
                })

            def emit_score(u, tk):
                # diagonal tiles (r >= 0) only have valid scores in their
                # last TT - 128*r columns; skip the fully-masked prefix.
                # In suffix-local coords the causal triangle is always the
                # first 128 columns; it gets a -1e30 PSUM preload so exp
                # masks for free.
                r = tk - 4 * qt
                off = 128 * r if r > 0 else 0
                w = TT - off
                q0 = qt * TT + off
                sps = psum.tile([128, TT], f32, tag="mm", name="sps")
                u["sps"][tk] = (sps, off, w)
                kblk = krot[:, tk * 128:(tk + 1) * 128]
                qh = qrot[u["h"]]
                if r >= 0:
                    # mask bias comes from the PE itself (has_written gotcha)
                    nc.tensor.matmul(sps[:, 0:128], mtri[:], mshift[:],
                                     start=True, stop=False)
                    nc.tensor.matmul(sps[:, 0:128], kblk,
                                     qh[:, q0:q0 + 128],
                                     start=False, stop=(w == 128))
                    if w > 128:
                        nc.tensor.matmul(sps[:, 128:w], kblk,
                                         qh[:, q0 + 128:q0 + w],
                                         start=False, stop=True)
                else:
                    nc.tensor.matmul(sps[:, :w], kblk, qh[:, q0:q0 + w],
                                     start=True, stop=True)

            def emit_exp(u, tk):
                sps, off, w = u["sps"][tk]
                e = epool.tile([128, TT], bf16, tag="e", name="e")
                nc.scalar.activation(e[:, :w], sps[:, :w],
                                     mybir.ActivationFunctionType.Exp,
                                     bias=0.0, scale=SCALE)
                u["sps"][tk] = None
                u["e"][tk] = (e, off, w)

            def emit_acc(u, tk):
                e, off, w = u["e"][tk]
                nc.tensor.matmul(u["dps"][:, off:], onesm[:], e[:, :w],
                                 start=(tk == 0), stop=(tk == ntk - 1))
                nc.tensor.matmul(u["yps"][:, off:], vtok[:, tk, :],
                                 e[:, :w],
                                 start=(tk == 0), stop=(tk == ntk - 1))
                u["e"][tk] = None

            for u in units:                      # prologue
                emit_score(u, 0)
                emit_exp(u, 0)
            for i in range(ntk):
                for u in units:
                    if i + 1 < ntk:
                        emit_score(u, i + 1)
                        emit_exp(u, i + 1)
                    emit_acc(u, i)
                pump()
            qs = slice(qt * TT, (qt + 1) * TT)
            for u in units:                      # epilogue
                # dps holds the denominator replicated in all partitions;
                # reciprocal + one fused normalize-and-store multiply
                drb = sb.tile([128, TT], f32, tag="drb", name="drb")
                nc.vector.reciprocal_approx_fast(drb[:], u["dps"][:])
                nc.vector.tensor_mul(yT[u["h"]][:, qs], u["yps"][:],
                                     drb[:])

        for qt in (3, 2, 1, 0):
            emit_pair(0, 1, qt)
            emit_pair(2, 3, qt)
            filler.extend(outproj_block(qt, m) for m in range(D // 128))
        while filler:
            filler.popleft()()

    nc.compile()
    return nc


def _host_shards(x, freqs_cis, wqkv, wo):
    import ml_dtypes
    bf16 = ml_dtypes.bfloat16

    # head-dim permutation: even dims then odd dims (for q and k only)
    perm = np.concatenate([np.arange(0, HD, 2), np.arange(1, HD, 2)])

    wq = wqkv[:Q_SIZE].reshape(NH, HD, D)[:, perm, :]
    wk = wqkv[Q_SIZE:Q_SIZE + KV_SIZE].reshape(NKV, HD, D)[:, perm, :]
    wv = wqkv[Q_SIZE + KV_SIZE:].reshape(NKV, HD, D)

    fr1 = np.ascontiguousarray(freqs_cis[:, :, 0].T, dtype=np.float32)
    fi1 = np.ascontiguousarray(freqs_cis[:, :, 1].T, dtype=np.float32)
    fr = np.vstack([fr1, fr1])
    # sign baked in so the rotary combine is a single add:
    #   rot[lo] = q_lo*fr + q_hi*(-fi) ; rot[hi] = q_hi*fr + q_lo*(+fi)
    fi = np.vstack([fi1, -fi1])

    # mask matmul operands: (mtri.T @ mshift)[p, c] = NEG iff p > c
    # (suffix-local causal bias for the diagonal 128x128 score block)
    mtri = np.triu(np.ones((128, 128), np.float32)).astype(bf16)
    mshift = (NEG * np.eye(128, k=-1, dtype=np.float32)).astype(bf16)

    in_maps = []
    for c in range(N_CORES):
        b, j = divmod(c, TPC)
        wshard = np.concatenate(
            [wq[TPC * j + h] for h in range(HEADS_PER_CORE)] +
            [wk[j], wv[j]], axis=0)                     # (768, D)
        # [chunk, p, ko, e] with d = ko*128 + p
        wpack = np.ascontiguousarray(
            wshard.reshape(HEADS_PER_CORE + 2, HD, N_KT, 128)
            .transpose(0, 3, 2, 1)).astype(bf16)
        in_maps.append({
            "xT": np.ascontiguousarray(x[b].T).astype(bf16),
            "wqkvT": wpack,
            "woT": np.ascontiguousarray(
                wo[:, j * E_LOC:(j + 1) * E_LOC].T).astype(bf16),
            "fr": fr,
            "fi": fi,
            "mtri": mtri,
            "mshift": mshift,
        })
    return in_maps


_NC_CACHE = {}


def _get_nc():
    if "nc" not in _NC_CACHE:
        _NC_CACHE["nc"] = _build_bass()
    return _NC_CACHE["nc"]


def kernel(x, freqs_cis, wqkv, wo, q_norm_w, k_norm_w, _want_results=False):
    # q_norm_w / k_norm_w are all-ones per the problem spec; rmsnorm weight
    # multiply is the identity and is folded away.
    from concourse.bass_utils import run_bass_kernel_spmd

    nc = _get_nc()
    in_maps = _host_shards(np.asarray(x, np.float32),
                           np.asarray(freqs_cis, np.float32),
                           np.asarray(wqkv, np.float32),
                           np.asarray(wo, np.float32))
    res = run_bass_kernel_spmd(nc, in_maps, core_ids=list(range(N_CORES)))
    parts = [r["outT"] for r in res.results]
    out = np.empty((B, S, D), np.float32)
    for b in range(B):
        acc = parts[TPC * b].astype(np.float32)
        for j in range(1, TPC):
            acc += parts[TPC * b + j].astype(np.float32)
        out[b] = acc.T
    if _want_results:
        return out, res
    return out
